# revision 8
# baseline (speedup 1.0000x reference)
"""Trainium2 Bass kernel for ContinuousMLP (B=1048576, 3->128->128->128->128->2, silu).

Strategy: pure data parallel over 8 NeuronCores (131072 rows each).
On-chip, activations live transposed: [feature (<=128 partitions), batch cols].
Per 512-col tile: 5 matmuls in float32r (1 cyc/row on PE vs 4 for fp32;
~1e-3 rel err), silu+bias fused on ScalarE in 2048-col groups (ScalarE is
the bottleneck: 4 sigmoid passes/elem @1.2GHz vs PE 5 passes @2.4GHz).
Layer-5 [2,512] outputs col-packed 4-per-PSUM-bank via tile_position; bias5
+ PSUM->SBUF copy on the otherwise-idle VectorE. Strided DMAs handle the
(B,2) <-> [2,N] transposes; SWDGE cast-DMAs round fp32 -> f32r on the way in.
"""

import numpy as np
from contextlib import ExitStack

import concourse.bass as bass
import concourse.tile as tile
from concourse import bacc, mybir
from concourse.bass_utils import run_bass_kernel_spmd

f32 = mybir.dt.float32
f32r = mybir.dt.float32r
SILU = mybir.ActivationFunctionType.Silu

N_CORES = 8
B_TOTAL = 1048576
BL = B_TOTAL // N_CORES  # 131072 rows per core
IN_DIM = 2
HID = 128
N_TILE = 512             # matmul moving-operand cols (1 PSUM bank fp32)
G = 4                    # tiles per activation group
FD = N_TILE * G          # 2048 cols per ScalarE activation op
N_GROUPS = BL // FD      # 64
CHUNK = 8192             # batch cols per input DMA chunk
GROUPS_PER_CHUNK = CHUNK // FD

_CACHE: dict = {}


def _build_nc():
    nc = bacc.Bacc("TRN2", target_bir_lowering=False, debug=False)

    x_d = nc.dram_tensor("xT", [IN_DIM, BL], f32, kind="ExternalInput").ap()
    t_d = nc.dram_tensor("t", [BL], f32, kind="ExternalInput").ap()
    w1_d = nc.dram_tensor("W1", [IN_DIM + 1, HID], f32, kind="ExternalInput").ap()
    w2_d = nc.dram_tensor("W2", [HID, HID], f32, kind="ExternalInput").ap()
    w3_d = nc.dram_tensor("W3", [HID, HID], f32, kind="ExternalInput").ap()
    w4_d = nc.dram_tensor("W4", [HID, HID], f32, kind="ExternalInput").ap()
    w5_d = nc.dram_tensor("W5", [HID, IN_DIM], f32, kind="ExternalInput").ap()
    b1_d = nc.dram_tensor("b1", [HID], f32, kind="ExternalInput").ap()
    b2_d = nc.dram_tensor("b2", [HID], f32, kind="ExternalInput").ap()
    b3_d = nc.dram_tensor("b3", [HID], f32, kind="ExternalInput").ap()
    b4_d = nc.dram_tensor("b4", [HID], f32, kind="ExternalInput").ap()
    # b5 pre-tiled host-side to one value per partition: b5rep[p] = b5[p % 2]
    b5r_d = nc.dram_tensor("b5rep", [HID], f32, kind="ExternalInput").ap()
    out_d = nc.dram_tensor("out", [IN_DIM, BL], f32, kind="ExternalOutput").ap()

    with tile.TileContext(nc) as tc, ExitStack() as ctx:
        singles = ctx.enter_context(tc.tile_pool(name="singles", bufs=1))
        in_pool = ctx.enter_context(tc.tile_pool(name="in3", bufs=2))
        h_pool = ctx.enter_context(tc.tile_pool(name="h", bufs=3))
        out_pool = ctx.enter_context(tc.tile_pool(name="osb", bufs=3))
        psum_pool = ctx.enter_context(tc.tile_pool(name="ps", bufs=2, space="PSUM"))

        # Weights as f32r (SWDGE cast-DMA rounds fp32 -> f32r; walrus requires
        # f32r matmul inputs to be produced rounded).
        w1r = singles.tile([IN_DIM + 1, HID], f32r)
        w2r = singles.tile([HID, HID], f32r)
        w3r = singles.tile([HID, HID], f32r)
        w4r = singles.tile([HID, HID], f32r)
        w5r = singles.tile([HID, IN_DIM], f32r)
        nc.gpsimd.dma_start(out=w1r[:], in_=w1_d[:])
        nc.gpsimd.dma_start(out=w2r[:], in_=w2_d[:])
        nc.gpsimd.dma_start(out=w3r[:], in_=w3_d[:])
        nc.gpsimd.dma_start(out=w4r[:], in_=w4_d[:])
        nc.gpsimd.dma_start(out=w5r[:], in_=w5_d[:])

        bs = []
        for bi, bd in enumerate((b1_d, b2_d, b3_d, b4_d, b5r_d)):
            bt = singles.tile([HID, 1], f32, tag=f"bias{bi}")
            nc.sync.dma_start(out=bt[:], in_=bd.rearrange("(n o) -> n o", o=1))
            bs.append(bt)
        b1s, b2s, b3s, b4s, b5s = bs
        w_layers = [w2r, w3r, w4r]
        b_layers = [b2s, b3s, b4s]

        for s in range(BL // CHUNK):
            in3 = in_pool.tile([IN_DIM + 1, CHUNK], f32r, tag="in3")
            nc.gpsimd.dma_start(
                out=in3[0:IN_DIM, :],
                in_=x_d[:, s * CHUNK:(s + 1) * CHUNK],
            )
            nc.gpsimd.dma_start(
                out=in3[IN_DIM:IN_DIM + 1, :],
                in_=t_d[s * CHUNK:(s + 1) * CHUNK].rearrange("(o n) -> o n", o=1),
            )
            for gi in range(GROUPS_PER_CHUNK):
                base = gi * FD
                gbase = s * CHUNK + base

                ps = psum_pool.tile([HID, FD], f32, tag="ps")
                for j in range(G):
                    nc.tensor.matmul(
                        ps[:, j * N_TILE:(j + 1) * N_TILE],
                        w1r[:],
                        in3[:, base + j * N_TILE: base + (j + 1) * N_TILE],
                        start=True, stop=True,
                    )
                h = h_pool.tile([HID, FD], f32r, tag="h")
                nc.scalar.activation(h[:], ps[:], SILU, bias=b1s[:, 0:1])

                for wl, bl in zip(w_layers, b_layers):
                    ps = psum_pool.tile([HID, FD], f32, tag="ps")
                    for j in range(G):
                        nc.tensor.matmul(
                            ps[:, j * N_TILE:(j + 1) * N_TILE],
                            wl[:],
                            h[:, j * N_TILE:(j + 1) * N_TILE],
                            start=True, stop=True,
                        )
                    h = h_pool.tile([HID, FD], f32r, tag="h")
                    nc.scalar.activation(h[:], ps[:], SILU, bias=bl[:, 0:1])

                # Layer 5: [2,512] per tile at partitions 0-1, one PSUM bank
                # per tile inside a 5th psum allocation.
                ps5 = psum_pool.tile([HID, FD], f32, tag="ps")
                for j in range(G):
                    nc.tensor.matmul(
                        ps5[0:IN_DIM, j * N_TILE:(j + 1) * N_TILE],
                        w5r[:],
                        h[:, j * N_TILE:(j + 1) * N_TILE],
                        start=True, stop=True,
                    )
                osb = out_pool.tile([IN_DIM, FD], f32, tag="osb")
                nc.vector.tensor_scalar_add(
                    osb[:], ps5[0:IN_DIM, :], b5s[0:IN_DIM, 0:1]
                )
                nc.sync.dma_start(
                    out=out_d[:, gbase:gbase + FD], in_=osb[:],
                )

    nc.compile()
    return nc


def _get_nc():
    if "nc" not in _CACHE:
        _CACHE["nc"] = _build_nc()
    return _CACHE["nc"]


def kernel(**inputs) -> np.ndarray:
    x = np.ascontiguousarray(np.asarray(inputs["x"], dtype=np.float32))
    t = np.ascontiguousarray(np.asarray(inputs["t"], dtype=np.float32))
    ws = {k: np.ascontiguousarray(np.asarray(inputs[k], dtype=np.float32))
          for k in ("W1", "W2", "W3", "W4", "W5", "b1", "b2", "b3", "b4")}
    b5 = np.asarray(inputs["b5"], dtype=np.float32)
    b5rep = np.tile(b5, HID // IN_DIM).astype(np.float32)

    nc = _get_nc()
    in_maps = []
    for c in range(N_CORES):
        m = {
            "xT": np.ascontiguousarray(x[c * BL:(c + 1) * BL].T),
            "t": t[c * BL:(c + 1) * BL],
            "b5rep": b5rep,
        }
        m.update(ws)
        in_maps.append(m)

    res = run_bass_kernel_spmd(nc, in_maps, list(range(N_CORES)))
    _CACHE["last_results"] = res
    out = np.concatenate(
        [res.results[c]["out"].T for c in range(N_CORES)], axis=0
    )
    return np.ascontiguousarray(out, dtype=np.float32)


# revision 10
# speedup vs baseline: 1.8612x; 1.8612x over previous
"""Trainium2 Bass kernel for ContinuousMLP (B=1048576, 3->128->128->128->128->2, silu).

Strategy: pure data parallel over 8 NeuronCores (131072 rows each).
On-chip, activations live transposed: [feature (<=128 partitions), batch cols].
Per 512-col tile: 5 matmuls in float32r (1 cyc/row on PE vs 4 for fp32;
~1e-3 rel err), silu+bias fused on ScalarE in 2048-col groups (ScalarE is
the bottleneck: 4 sigmoid passes/elem @1.2GHz vs PE 5 passes @2.4GHz).
Layer-5 [2,512] outputs col-packed 4-per-PSUM-bank via tile_position; bias5
+ PSUM->SBUF copy on the otherwise-idle VectorE. Strided DMAs handle the
(B,2) <-> [2,N] transposes; SWDGE cast-DMAs round fp32 -> f32r on the way in.
"""

import numpy as np
from contextlib import ExitStack

import concourse.bass as bass
import concourse.tile as tile
from concourse import bacc, mybir
from concourse.bass_utils import run_bass_kernel_spmd

f32 = mybir.dt.float32
f32r = mybir.dt.float32r
SILU = mybir.ActivationFunctionType.Silu

N_CORES = 8
B_TOTAL = 1048576
BL = B_TOTAL // N_CORES  # 131072 rows per core
IN_DIM = 2
HID = 128
N_TILE = 512             # matmul moving-operand cols (1 PSUM bank fp32)
G = 4                    # tiles per activation group
FD = N_TILE * G          # 2048 cols per ScalarE activation op
N_GROUPS = BL // FD      # 64
CHUNK = 8192             # batch cols per input DMA chunk
GROUPS_PER_CHUNK = CHUNK // FD

_CACHE: dict = {}


def _build_nc():
    nc = bacc.Bacc("TRN2", target_bir_lowering=False, debug=False)

    x_d = nc.dram_tensor("xT", [IN_DIM, BL], f32, kind="ExternalInput").ap()
    t_d = nc.dram_tensor("t", [BL], f32, kind="ExternalInput").ap()
    w1_d = nc.dram_tensor("W1", [IN_DIM + 1, HID], f32, kind="ExternalInput").ap()
    w2_d = nc.dram_tensor("W2", [HID, HID], f32, kind="ExternalInput").ap()
    w3_d = nc.dram_tensor("W3", [HID, HID], f32, kind="ExternalInput").ap()
    w4_d = nc.dram_tensor("W4", [HID, HID], f32, kind="ExternalInput").ap()
    w5_d = nc.dram_tensor("W5", [HID, IN_DIM], f32, kind="ExternalInput").ap()
    b1_d = nc.dram_tensor("b1", [HID], f32, kind="ExternalInput").ap()
    b2_d = nc.dram_tensor("b2", [HID], f32, kind="ExternalInput").ap()
    b3_d = nc.dram_tensor("b3", [HID], f32, kind="ExternalInput").ap()
    b4_d = nc.dram_tensor("b4", [HID], f32, kind="ExternalInput").ap()
    # b5 pre-tiled host-side to one value per partition: b5rep[p] = b5[p % 2]
    b5r_d = nc.dram_tensor("b5rep", [HID], f32, kind="ExternalInput").ap()
    out_d = nc.dram_tensor("out", [IN_DIM, BL], f32, kind="ExternalOutput").ap()

    with tile.TileContext(nc) as tc, ExitStack() as ctx:
        singles = ctx.enter_context(tc.tile_pool(name="singles", bufs=1))
        in_pool = ctx.enter_context(tc.tile_pool(name="in3", bufs=2))
        h_pool = ctx.enter_context(tc.tile_pool(name="h", bufs=4))
        out_pool = ctx.enter_context(tc.tile_pool(name="osb", bufs=3))
        psum_pool = ctx.enter_context(tc.tile_pool(name="ps", bufs=2, space="PSUM"))

        # Weights as f32r (SWDGE cast-DMA rounds fp32 -> f32r; walrus requires
        # f32r matmul inputs to be produced rounded).
        w1r = singles.tile([IN_DIM + 1, HID], f32r)
        w2r = singles.tile([HID, HID], f32r)
        w3r = singles.tile([HID, HID], f32r)
        w4r = singles.tile([HID, HID], f32r)
        w5r = singles.tile([HID, IN_DIM], f32r)
        nc.gpsimd.dma_start(out=w1r[:], in_=w1_d[:])
        nc.gpsimd.dma_start(out=w2r[:], in_=w2_d[:])
        nc.gpsimd.dma_start(out=w3r[:], in_=w3_d[:])
        nc.gpsimd.dma_start(out=w4r[:], in_=w4_d[:])
        nc.gpsimd.dma_start(out=w5r[:], in_=w5_d[:])

        bs = []
        for bi, bd in enumerate((b1_d, b2_d, b3_d, b4_d, b5r_d)):
            bt = singles.tile([HID, 1], f32, tag=f"bias{bi}")
            nc.sync.dma_start(out=bt[:], in_=bd.rearrange("(n o) -> n o", o=1))
            bs.append(bt)
        b1s, b2s, b3s, b4s, b5s = bs
        w_layers = [w2r, w3r, w4r]
        b_layers = [b2s, b3s, b4s]

        # Two-stream software pipeline: interleave even/odd groups so the PE
        # always has the other stream's matmuls while ScalarE runs silu.
        # Each stream holds ONE [128, FD] psum tile for its whole group
        # (layers reuse it serially; the skew guarantees the ACT read is
        # done before the next layer's matmuls overwrite it). MM5 writes
        # [2,512]-per-bank into the same tile after ACT4 reads it.
        # 2 streams x 4 banks = all 8 PSUM banks.

        def load_chunk(s):
            in3 = in_pool.tile([IN_DIM + 1, CHUNK], f32r, tag="in3")
            nc.gpsimd.dma_start(
                out=in3[0:IN_DIM, :],
                in_=x_d[:, s * CHUNK:(s + 1) * CHUNK],
            )
            nc.gpsimd.dma_start(
                out=in3[IN_DIM:IN_DIM + 1, :],
                in_=t_d[s * CHUNK:(s + 1) * CHUNK].rearrange("(o n) -> o n", o=1),
            )
            return in3

        def mm4(ps, w, rhs, roff):
            for j in range(G):
                nc.tensor.matmul(
                    ps[:, j * N_TILE:(j + 1) * N_TILE],
                    w[:],
                    rhs[:, roff + j * N_TILE: roff + (j + 1) * N_TILE],
                    start=True, stop=True,
                )

        def act(ps, bias):
            h = h_pool.tile([HID, FD], f32r, tag="h")
            nc.scalar.activation(h[:], ps[:], SILU, bias=bias[:, 0:1])
            return h

        class Stream:
            pass

        chunks = {}

        def start_group(g):
            st = Stream()
            s, gi = divmod(g, GROUPS_PER_CHUNK)
            if gi == 0:
                chunks[s] = load_chunk(s)
            st.in3 = chunks[s]
            st.base = gi * FD
            st.gbase = g * FD
            st.ps = psum_pool.tile([HID, FD], f32, tag="ps")
            return st

        def phase(st, li):
            # li = 0..3 hidden layers, 4 = output layer + epilogue
            if li == 0:
                mm4(st.ps, w1r, st.in3, st.base)
                st.h = act(st.ps, b1s)
            elif li < 4:
                mm4(st.ps, w_layers[li - 1], st.h, 0)
                st.h = act(st.ps, b_layers[li - 1])
            else:
                for j in range(G):
                    nc.tensor.matmul(
                        st.ps[0:IN_DIM, j * N_TILE:(j + 1) * N_TILE],
                        w5r[:],
                        st.h[:, j * N_TILE:(j + 1) * N_TILE],
                        start=True, stop=True,
                    )
                osb = out_pool.tile([IN_DIM, FD], f32, tag="osb")
                nc.vector.tensor_scalar_add(
                    osb[:], st.ps[0:IN_DIM, :], b5s[0:IN_DIM, 0:1]
                )
                nc.sync.dma_start(
                    out=out_d[:, st.gbase:st.gbase + FD], in_=osb[:],
                )

        for k in range(N_GROUPS // 2):
            sa = start_group(2 * k)
            sb = start_group(2 * k + 1)
            for li in range(5):
                phase(sa, li)
                phase(sb, li)

    nc.compile()
    return nc


def _get_nc():
    if "nc" not in _CACHE:
        _CACHE["nc"] = _build_nc()
    return _CACHE["nc"]


def kernel(**inputs) -> np.ndarray:
    x = np.ascontiguousarray(np.asarray(inputs["x"], dtype=np.float32))
    t = np.ascontiguousarray(np.asarray(inputs["t"], dtype=np.float32))
    ws = {k: np.ascontiguousarray(np.asarray(inputs[k], dtype=np.float32))
          for k in ("W1", "W2", "W3", "W4", "W5", "b1", "b2", "b3", "b4")}
    b5 = np.asarray(inputs["b5"], dtype=np.float32)
    b5rep = np.tile(b5, HID // IN_DIM).astype(np.float32)

    nc = _get_nc()
    in_maps = []
    for c in range(N_CORES):
        m = {
            "xT": np.ascontiguousarray(x[c * BL:(c + 1) * BL].T),
            "t": t[c * BL:(c + 1) * BL],
            "b5rep": b5rep,
        }
        m.update(ws)
        in_maps.append(m)

    res = run_bass_kernel_spmd(nc, in_maps, list(range(N_CORES)))
    _CACHE["last_results"] = res
    out = np.concatenate(
        [res.results[c]["out"].T for c in range(N_CORES)], axis=0
    )
    return np.ascontiguousarray(out, dtype=np.float32)
